# revision 21
# baseline (speedup 1.0000x reference)
"""Trainium2 Bass kernel for a 12-head attention block.

Problem (hardcoded): x [16, 1024, 768] f32, w_qkv [2304, 768], w_proj
[768, 768], b_proj [768].  out = proj(softmax(q k^T / sqrt(64)) v).

Sharding: pure data parallel over batch - 16 batches / 8 cores = 2
batches per core, no collectives.  All layout transposes happen on the
host: each core receives x^T per batch and produces out^T per batch.

v3 design (vs v2):
  * HEAD-PAIR CONCURRENT QK^T: the S^T matmuls have contraction K=64
    (head_dim), which occupies only half the 128-row PE array.  The PE
    runs matmuls with disjoint 32-row-groups CONCURRENTLY (measured
    2.24x on HW for adjacent K=64 MMs on rows 0-63 / 64-127).  Heads
    2m (rows 0-63 of the qk tile) and 2m+1 (rows 64-127) are emitted
    as adjacent matmul pairs into separate PSUM tiles, halving the
    effective S^T cost from 98304 to ~49152 cycles/batch.
  * per-jc PV for the even head (o accums in 2 dedicated PSUM banks);
    the odd head's exp outputs are buffered in SBUF and its PV runs as
    8-matmul accumulation chains scheduled like fillers during the next
    pair (sharing the 2 filler PSUM banks).
  * PSUM: 2x s-pair tiles [128,1024] (4 banks) + 2x o_A [65,512]
    (2 banks) + 2x shared filler/B-chain [128,512] (2 banks) = 8.
  * all matmul operands bf16 (k stationary fp8e4); PSUM f32.
  * cross-batch software pipelining as v2: batch b+1 qkv projection and
    batch b out-projection run as paced fillers inside attention
    windows so the PE never waits on the exp (ACT) latency.
"""

import numpy as np
from contextlib import ExitStack

import ml_dtypes

import concourse.bass as bass
import concourse.mybir as mybir
import concourse.tile as tile
from concourse import bacc
from concourse import bass_utils

F32 = mybir.dt.float32
BF16 = mybir.dt.bfloat16
FP8 = mybir.dt.float8e4
EXP = mybir.ActivationFunctionType.Exp

B, N, C = 16, 1024, 768
H, D = 12, 64
E = 3 * C
NCORES = 8
BL = B // NCORES          # batches per core
T = BL * N                # tokens per core
KC = C // 128             # feature chunks of 128
JC = N // 128             # token chunks of 128
SCALE = float(D) ** -0.5

_CACHE = {}


def _mm(nc, out, lhsT, rhs, **kw):
    nc.tensor.matmul(out, lhsT=lhsT, rhs=rhs, **kw)


def _build(ctx, tc):
    nc = tc.nc
    dram = ctx.enter_context(tc.tile_pool(name="dram", bufs=1, space="DRAM"))
    # x^T blocked: [kc, b, 128, N] bf16, each per-batch chunk contiguous
    xT_d = dram.tile([KC, BL, 128, N], BF16, kind="ExternalInput", name="xTb", uniquify=False)
    # q/k weights j-major: [j, p, kc*128+c] so one j-output-chunk (used by
    # one qk_unit across all kc) is a single contiguous-row DMA
    wqkj_d = dram.tile([2 * KC, 128, C], BF16, kind="ExternalInput", name="wqkjb", uniquify=False)
    # v weights per-kc slabs [kc, 128, 768] bf16
    wv_d = dram.tile([KC, 128, C], BF16, kind="ExternalInput", name="wvb", uniquify=False)
    # w_proj^T per-kc slabs [kc, 128, 768] bf16
    wproj_d = dram.tile([KC, 128, C], BF16, kind="ExternalInput", name="wprojb", uniquify=False)
    bproj_d = dram.tile([C, 1], F32, kind="ExternalInput", name="bproj", uniquify=False)
    ident_d = dram.tile([128, 128], BF16, kind="ExternalInput", name="ident", uniquify=False)
    # out^T blocked: [oc, b, 128, N] f32
    outT_d = dram.tile([KC, BL, 128, N], F32, kind="ExternalOutput", name="outTb", uniquify=False)

    consts = ctx.enter_context(tc.tile_pool(name="consts", bufs=1))
    wqk_pool = ctx.enter_context(tc.tile_pool(name="wqk", bufs=2 * (KC - 1)))
    wqk0_pool = ctx.enter_context(tc.tile_pool(name="wqk0", bufs=2 * KC))
    wv_pool = ctx.enter_context(tc.tile_pool(name="wv", bufs=KC))
    wp_pool = ctx.enter_context(tc.tile_pool(name="wproj", bufs=KC))
    xt_pool = ctx.enter_context(tc.tile_pool(name="xt", bufs=2 * KC))
    q_pool = ctx.enter_context(tc.tile_pool(name="qp", bufs=KC + 2))
    k_pool = ctx.enter_context(tc.tile_pool(name="kp", bufs=KC + 2))
    va_pool = ctx.enter_context(tc.tile_pool(name="va", bufs=2 * JC))
    ot_pool = ctx.enter_context(tc.tile_pool(name="ot", bufs=2 * KC))
    pp_pool = ctx.enter_context(tc.tile_pool(name="pp", bufs=21))
    sm_pool = ctx.enter_context(tc.tile_pool(name="small", bufs=8))
    lb_pool = ctx.enter_context(tc.tile_pool(name="lb", bufs=4))
    ob_pool = ctx.enter_context(tc.tile_pool(name="ob", bufs=2))
    # PSUM: s tiles are hf-interleaved head pairs [A-half | B-half]
    # [128, 1024] (2 banks each, 2 bufs): the two MMs writing one tile
    # are gated on the SAME recycle event (-> atomic concurrent pair)
    # while the two tiles double-buffer across i-halves/slots (-> the
    # exp latency pipelines).  + o_A accums (2 x 1 bank) + shared
    # filler / B-chain pool (2 x 1 bank) = 8 banks.
    ps_s = ctx.enter_context(tc.tile_pool(name="ps_s", bufs=2, space="PSUM"))
    po_pool = ctx.enter_context(tc.tile_pool(name="po", bufs=2, space="PSUM"))
    ps_a = ctx.enter_context(tc.tile_pool(name="ps_a", bufs=2, space="PSUM"))

    bias_sb = consts.tile([128, KC], F32)
    nc.sync.dma_start(
        out=bias_sb, in_=bproj_d[:, 0].rearrange("(k p) -> p k", p=128)
    )
    ident_sb = consts.tile([128, 128], BF16)
    nc.sync.dma_start(out=ident_sb, in_=ident_d)

    # ---- input DMA, ordered by first use ----
    xt = {}   # (b, kc) -> [128, N] bf16
    wqk_t, wv = {}, []

    def load_wqkj(j, split=1):
        t = wqk_pool.tile([128, KC, 128], BF16, name=f"wqkj{j}", tag="wqk")
        step = 128 // split
        for p in range(0, 128, step):
            nc.sync.dma_start(
                out=t[p:p + step, :, :].rearrange("p a b -> p (a b)"),
                in_=wqkj_d[j, p:p + step, :])
        wqk_t[j] = t

    # mt=0 k/q weights as per-kc tiles interleaved with batch-0 x chunks:
    # the first qk accumulation chain starts ~2us in (DMA-paced) instead
    # of waiting ~13us for monolithic weight tiles.
    wqk0 = {}
    for kc in range(KC):
        t = wqk0_pool.tile([128, 128], BF16, name=f"wqk0k_{kc}", tag="w0")
        nc.sync.dma_start(out=t, in_=wqkj_d[KC, :, kc * 128:(kc + 1) * 128])
        wqk0[(KC, kc)] = t
        xt[(0, kc)] = xt_pool.tile([128, N], BF16, name=f"xt0_{kc}", tag="xt")
        nc.sync.dma_start(out=xt[(0, kc)], in_=xT_d[kc, 0])
        t = wqk0_pool.tile([128, 128], BF16, name=f"wqk0q_{kc}", tag="w0")
        nc.sync.dma_start(out=t, in_=wqkj_d[0, :, kc * 128:(kc + 1) * 128])
        wqk0[(0, kc)] = t
    for kc in range(KC):
        wvt = wv_pool.tile([128, KC, 128], BF16, name=f"wv{kc}", tag="wv")
        nc.sync.dma_start(out=wvt.rearrange("p a b -> p (a b)"), in_=wv_d[kc])
        wv.append(wvt)
    for mt in range(1, KC):
        load_wqkj(KC + mt)
        load_wqkj(mt)
    wqk = {(j, kc): wqk_t[j][:, kc, :]
           for j in list(range(1, KC)) + list(range(KC + 1, 2 * KC))
           for kc in range(KC)}
    wqk.update(wqk0)
    wp = {}
    for kc in range(KC):
        t = wp_pool.tile([128, KC, 128], BF16, name=f"wpk{kc}", tag="wp")
        nc.sync.dma_start(out=t.rearrange("p a b -> p (a b)"), in_=wproj_d[kc])
        for oc in range(KC):
            wp[(kc, oc)] = t[:, oc, :]
    # batch-1 x after the weights
    for kc in range(KC):
        xt[(1, kc)] = xt_pool.tile([128, N], BF16, name=f"xt1_{kc}", tag="xt")
        nc.sync.dma_start(out=xt[(1, kc)], in_=xT_d[kc, 1])

    qt = {}   # (b, mt) -> [128, N] bf16   q features, head-pair mt
    kt = {}   # (b, mt) -> [128, N] fp8
    va = {}   # (b, jc) -> [128, H, D+1] bf16
    ot = {}   # (b, mt) -> [128, N] bf16

    # ---- phase A units: qkv projection for batch b, as closure LISTS
    # (sub-unit granularity ~1-1.3us for smooth filler pacing) ----
    def phase_a_units(b):
        units = {}

        def v_unit(jc):
            vat_box = {}

            def get_vat():
                if "t" not in vat_box:
                    vat_box["t"] = va_pool.tile(
                        [128, H, D + 1], BF16, name=f"va{b}_{jc}", tag="va")
                    va[(b, jc)] = vat_box["t"]
                return vat_box["t"]

            def emit_a():
                vat = get_vat()
                xs = [xt[(b, kc)][:, jc * 128:(jc + 1) * 128] for kc in range(KC)]
                wvf = [wv[kc].rearrange("p a b -> p (a b)") for kc in range(KC)]
                vps_a = ps_a.tile([128, 512], F32, name=f"vpsa{b}_{jc}", tag="a")
                for kc in range(KC):
                    _mm(nc, vps_a, xs[kc], wvf[kc][:, 0:512],
                        start=(kc == 0), stop=(kc == KC - 1))
                nc.vector.tensor_copy(
                    out=vat[:, 0:8, 0:D],
                    in_=vps_a.rearrange("p (h d) -> p h d", h=8),
                )

            def emit_b():
                vat = get_vat()
                xs = [xt[(b, kc)][:, jc * 128:(jc + 1) * 128] for kc in range(KC)]
                wvf = [wv[kc].rearrange("p a b -> p (a b)") for kc in range(KC)]
                vps_b = ps_a.tile([128, 512], F32, name=f"vpsb{b}_{jc}", tag="a")
                for kc in range(KC):
                    _mm(nc, vps_b[:, 0:256], xs[kc], wvf[kc][:, 512:C],
                        start=(kc == 0), stop=(kc == KC - 1))
                nc.vector.tensor_copy(
                    out=vat[:, 8:H, 0:D],
                    in_=vps_b[:, 0:256].rearrange("p (h d) -> p h d", h=4),
                )
                nc.vector.memset(vat[:, :, D:D + 1], 1.0)
            return [emit_a, emit_b]

        def qk_unit(which, mt):
            dest_box = {}

            def get_dest():
                if "t" not in dest_box:
                    # k (which=1) is only a stationary operand of QK^T:
                    # fp8e4 halves SBUF; perturbs logits ~0.6% on P.
                    if which == 1:
                        dest_box["t"] = k_pool.tile([128, N], FP8, name=f"k{b}_{mt}", tag="kp")
                        kt[(b, mt)] = dest_box["t"]
                    else:
                        dest_box["t"] = q_pool.tile([128, N], BF16, name=f"q{b}_{mt}", tag="qp")
                        qt[(b, mt)] = dest_box["t"]
                return dest_box["t"]

            def emit_hf(hf):
                def emit():
                    dest = get_dest()
                    ps = ps_a.tile([128, 512], F32, name=f"ps{b}_{which}_{mt}_{hf}", tag="a")
                    for kc in range(KC):
                        w = wqk[(which * KC + mt, kc)]
                        _mm(nc, ps,
                            w, xt[(b, kc)][:, hf * 512:(hf + 1) * 512],
                            start=(kc == 0), stop=(kc == KC - 1))
                    nc.vector.tensor_copy(out=dest[:, hf * 512:(hf + 1) * 512], in_=ps)
                return emit
            return [emit_hf(0), emit_hf(1)]

        for jc in range(JC):
            units[("v", jc)] = v_unit(jc)
        for mt in range(KC):
            units[("k", mt)] = qk_unit(1, mt)
            units[("q", mt)] = qk_unit(0, mt)
        return units

    # ---- normalize one [65, 512] accumulator half into ot rows ----
    def norm_half(b, mt, off, hf, o_ps, tag):
        l_sb = sm_pool.tile([1, 512], F32, name=f"l{tag}", tag="sm")
        nc.vector.tensor_copy(out=l_sb, in_=o_ps[D:D + 1, :])
        nc.vector.reciprocal_approx_fast(out=l_sb, in_=l_sb)
        lb = lb_pool.tile([D, 512], F32, name=f"lb{tag}", tag="lb")
        nc.gpsimd.partition_broadcast(lb, l_sb, channels=D)
        nc.vector.tensor_mul(
            out=ot[(b, mt)][off:off + D, hf * 512:(hf + 1) * 512],
            in0=o_ps[0:D, :], in1=lb,
        )

    # ---- attention for batch b: head-pair loops with paced fillers ----
    # fillers: list of dicts {dl, nb, w, fn} (deadline point, not-before
    # point, PE-ns weight).  48 pace points per batch (pair mt x jc).
    def attention_pairs(b, fillers):
        fillers.sort(key=lambda f: f["dl"])
        total_w = sum(f["w"] for f in fillers)
        state = {"w": 0.0}
        npace = KC * JC
        # pair-boundary points need ~2.2us of filler coverage (the last
        # slot's exps gate EVERYTHING pair-local: next S, PV, chains),
        # mid-pair slots much less -> nonlinear pacing budget
        pw = [2.2 if p % JC == 0 else (1.6 if p % JC == 7 else 0.75)
              for p in range(npace)]
        cw = []
        acc = 0.0
        for p in range(npace):
            acc += pw[p]
            cw.append(acc)
        for p in range(npace):
            cw[p] *= total_w / acc

        def insert(item):
            # keep sorted by deadline
            i = 0
            while i < len(fillers) and fillers[i]["dl"] <= item["dl"]:
                i += 1
            fillers.insert(i, item)

        def pace(point):
            budget = cw[point]
            while True:
                pick = None
                for i, f in enumerate(fillers):
                    if f["nb"] <= point and (state["w"] < budget or f["dl"] <= point):
                        pick = i
                        break
                    if f["dl"] > point and state["w"] >= budget:
                        break
                if pick is None:
                    break
                f = fillers.pop(pick)
                state["w"] += f["w"]
                f["fn"]()

        def b_chain(mt, hf, pt_map):
            def emit():
                o_ps = ps_a.tile([D + 1, 512], F32, name=f"obch{b}_{mt}_{hf}", tag="a")
                for j in range(JC):
                    _mm(nc, o_ps,
                        va[(b, j)][:, 2 * mt + 1, :],
                        pt_map[(j, hf)][:, 512:N],
                        start=(j == 0), stop=(j == JC - 1))
                norm_half(b, mt, D, hf, o_ps, f"B{b}_{mt}_{hf}")
            return emit

        for mt in range(KC):
            hA = 2 * mt
            kt_t, qt_t = kt[(b, mt)], qt[(b, mt)]
            o_A = [po_pool.tile([D + 1, 512], F32, name=f"oA{b}_{mt}_{hf}", tag="ops")
                   for hf in range(2)]
            # batch 1 last pair: fillers are flushed by then, so the ps_a
            # banks are free to hold per-jc B accumulators -> no serial
            # chain+norm tail before the final out-projection
            o_B = None
            if b == 1 and mt == KC - 1:
                o_B = [ps_a.tile([D + 1, 512], F32, name=f"oB{b}_{hf}", tag="a")
                       for hf in range(2)]
            ot[(b, mt)] = ot_pool.tile([128, N], BF16, name=f"ot{b}_{mt}", tag="ot")
            pts = {}
            for jc in range(JC + 1):
                point = mt * JC + jc
                # fillers FIRST in the slot: the greedy tile scheduler then
                # spends PE time on them while ACT finishes exp_B(jc-1), so
                # both s tiles of the pair are ready when the PE reaches
                # them and the pair MMs stay adjacent (-> concurrent).
                pace(min(point, npace - 1))
                if jc < JC:
                    js = slice(jc * 128, (jc + 1) * 128)
                    for hf in range(2):
                        s_t = ps_s.tile([128, N], F32,
                                        name=f"s{b}_{mt}_{jc}_{hf}", tag="s")
                        hs = slice(hf * 512, (hf + 1) * 512)
                        _mm(nc, s_t[:, 0:512], kt_t[0:D, js], qt_t[0:D, hs])
                        _mm(nc, s_t[:, 512:N], kt_t[D:128, js], qt_t[D:128, hs])
                        pt = pp_pool.tile([128, N], BF16,
                                          name=f"pt{b}_{mt}_{jc}_{hf}", tag="pp")
                        nc.scalar.activation(out=pt, in_=s_t, func=EXP, scale=SCALE)
                        pts[(jc, hf)] = pt
                if jc > 0:
                    j = jc - 1
                    for hf in range(2):
                        _mm(nc, o_A[hf],
                            va[(b, j)][:, hA, :],
                            pts[(j, hf)][:, 0:512],
                            start=(j == 0), stop=(j == JC - 1))
                    if o_B is not None:
                        for hf in range(2):
                            _mm(nc, o_B[hf],
                                va[(b, j)][:, hA + 1, :],
                                pts[(j, hf)][:, 512:N],
                                start=(j == 0), stop=(j == JC - 1))
            # normalize even head (rows 0..63 of ot)
            for hf in range(2):
                norm_half(b, mt, 0, hf, o_A[hf], f"A{b}_{mt}_{hf}")
            # odd head PV: 2 accumulation chains over buffered pt_B,
            # scheduled as fillers during the next pair (tail: inline)
            if o_B is None:
                b_chain(mt, 0, pts)()
                b_chain(mt, 1, pts)()
            else:
                for hf in range(2):
                    norm_half(b, mt, D, hf, o_B[hf], f"B{b}_{mt}_{hf}")
        # drain any remaining fillers
        for f in fillers:
            f["fn"]()

    # ---- output projection units for batch b ----
    def outproj_units(b):
        units = []
        obs = {}

        def half_unit(oc, hf):
            def emit():
                if oc not in obs:
                    obs[oc] = ob_pool.tile([128, N], F32, name=f"ob{b}_{oc}", tag="ob")
                pps = ps_a.tile([128, 512], F32, name=f"pps{b}_{oc}_{hf}", tag="a")
                for kc in range(KC):
                    _mm(nc, pps,
                        wp[(kc, oc)],
                        ot[(b, kc)][:, hf * 512:(hf + 1) * 512],
                        start=(kc == 0), stop=(kc == KC - 1))
                nc.vector.tensor_scalar_add(
                    out=obs[oc][:, hf * 512:(hf + 1) * 512],
                    in0=pps, scalar1=bias_sb[:, oc:oc + 1])
                if hf == 1:
                    nc.sync.dma_start(out=outT_d[oc, b], in_=obs[oc])
            return emit

        for oc in range(KC):
            for hf in range(2):
                units.append(half_unit(oc, hf))
        return units

    # two-stage out-projection for batch 1: partial (kc 0..3) runs as
    # window-1 fillers once those head-pairs are normalized; the tail
    # only does the kc4+kc5 accumulation + fused (partial + bias) add.
    KPART = 4

    def outproj1_partials():
        parts = {}
        units = []

        def part_unit(oc):
            def emit():
                pt = xt_pool.tile([128, N], BF16, name=f"op1p{oc}", tag="xt")
                parts[oc] = pt
                for hf in range(2):
                    pps = ps_a.tile([128, 512], F32, name=f"pp1{oc}_{hf}", tag="a")
                    for kc in range(KPART):
                        _mm(nc, pps,
                            wp[(kc, oc)],
                            ot[(1, kc)][:, hf * 512:(hf + 1) * 512],
                            start=(kc == 0), stop=(kc == KPART - 1))
                    nc.vector.tensor_copy(
                        out=pt[:, hf * 512:(hf + 1) * 512], in_=pps)
            return emit

        for oc in range(KC):
            units.append(part_unit(oc))
        return parts, units

    def outproj1_finals(parts):
        for oc in range(KC):
            ob = ob_pool.tile([128, N], F32, name=f"ob1f{oc}", tag="ob")
            for hf in range(2):
                pps = ps_a.tile([128, 512], F32, name=f"pf1{oc}_{hf}", tag="a")
                for kc in range(KPART, KC):
                    _mm(nc, pps,
                        wp[(kc, oc)],
                        ot[(1, kc)][:, hf * 512:(hf + 1) * 512],
                        start=(kc == KPART), stop=False)
                # partial re-enters via an identity matmul accumulation
                _mm(nc, pps, ident_sb,
                    parts[oc][:, hf * 512:(hf + 1) * 512],
                    start=False, stop=True)
                nc.vector.tensor_scalar_add(
                    out=ob[:, hf * 512:(hf + 1) * 512],
                    in0=pps, scalar1=bias_sb[:, oc:oc + 1])
            nc.sync.dma_start(out=outT_d[oc, 1], in_=ob)

    # ---- pipeline ----
    a0 = phase_a_units(0)
    a1 = phase_a_units(1)
    LAST = KC * JC - 1
    # prologue: minimum to start pair 0 + early v units
    for key in [("k", 0), ("q", 0)] + [("v", jc) for jc in range(3)]:
        for sub in a0[key]:
            sub()
    fill0 = []

    def add(fl, key, units, dl, nb, w):
        for sub in units[key]:
            fl.append({"dl": dl, "nb": nb, "w": w, "fn": sub})

    for jc in range(3, JC):
        add(fill0, ("v", jc), a0, jc - 1, 0, 960)
    for mt in range(1, KC):
        dl = mt * JC - 1
        add(fill0, ("k", mt), a0, dl, 0, 1280)
        add(fill0, ("q", mt), a0, dl, 0, 1280)
    for jc in range(3):
        add(fill0, ("v", jc), a1, 41 + jc, 30, 960)
    for w_ in ("k", "q"):
        add(fill0, (w_, 0), a1, 45, 30, 1280)
    attention_pairs(0, fill0)

    fill1 = []
    for jc in range(3, JC):
        add(fill1, ("v", jc), a1, jc - 1, 0, 960)
    for mt in range(1, KC):
        # staggered so completion precedes pair mt's first use
        ksub = a1[("k", mt)]
        qsub = a1[("q", mt)]
        fill1.append({"dl": mt * JC - 4, "nb": 0, "w": 1280, "fn": qsub[0]})
        fill1.append({"dl": mt * JC - 4, "nb": 0, "w": 1280, "fn": qsub[1]})
        fill1.append({"dl": mt * JC - 4, "nb": 0, "w": 1280, "fn": ksub[0]})
        fill1.append({"dl": min(mt * JC + 1, 39), "nb": 0, "w": 1280, "fn": ksub[1]})
    for i, u in enumerate(outproj_units(0)):
        # pinned near pair boundaries: guaranteed PE food while the last
        # exps of each pair gate all pair-local work
        dl = min(((i // 2) + 1) * JC - 1, 39)
        fill1.append({"dl": dl, "nb": max(dl - 3, 0), "w": 1280, "fn": u})
    op1_parts, op1_units = outproj1_partials()
    for i, u in enumerate(op1_units):
        fill1.append({"dl": min(33 + i, 39), "nb": 33, "w": 2150, "fn": u})
    attention_pairs(1, fill1)
    outproj1_finals(op1_parts)


def get_nc():
    if "nc" not in _CACHE:
        nc = bacc.Bacc(None, target_bir_lowering=False, debug=False)
        with tile.TileContext(nc) as tc:
            with ExitStack() as ctx:
                _build(ctx, tc)
        nc.compile()
        _CACHE["nc"] = nc
    return _CACHE["nc"]


def make_in_maps(x, w_qkv, w_proj, b_proj):
    x = np.asarray(x, dtype=np.float32)
    w_qkv = np.asarray(w_qkv, dtype=np.float32)
    w_proj = np.asarray(w_proj, dtype=np.float32)
    # q/k weights j-major: wqkjb[j, p, kc*128+c2] = w_qkv[j*128+c2, kc*128+p]
    wqkj = np.ascontiguousarray(
        w_qkv[0:2 * C].reshape(2 * KC, 128, KC, 128).transpose(0, 3, 2, 1)
        .reshape(2 * KC, 128, C)
    ).astype(ml_dtypes.bfloat16)
    # v weights per-kc slabs [kc, 128p, 768]: w_qkv^T[c, 2C:3C]
    wvb = np.ascontiguousarray(
        w_qkv[2 * C:3 * C].T.reshape(KC, 128, C)
    ).astype(ml_dtypes.bfloat16)
    # w_proj^T [c, o] -> per-kc slabs [kc, 128, 768] bf16
    wprojb = np.ascontiguousarray(w_proj.T.reshape(KC, 128, C)).astype(ml_dtypes.bfloat16)
    bp = np.ascontiguousarray(b_proj.astype(np.float32).reshape(C, 1))
    ident = np.eye(128, dtype=np.float32).astype(ml_dtypes.bfloat16)
    in_maps = []
    for c in range(NCORES):
        # x^T [c, t] -> blocks [kc, b, 128, N] bf16
        xT = x[c * BL:(c + 1) * BL].reshape(T, C).T  # [768, 2048]
        xb = np.ascontiguousarray(
            xT.reshape(KC, 128, BL, N).transpose(0, 2, 1, 3)
        ).astype(ml_dtypes.bfloat16)
        in_maps.append({"xTb": xb, "wqkjb": wqkj, "wvb": wvb,
                        "wprojb": wprojb, "bproj": bp, "ident": ident})
    return in_maps


def assemble_out(results):
    outs = []
    for c in range(NCORES):
        ob = results[c]["outTb"]  # [oc, b, 128, N]
        oT = ob.transpose(0, 2, 1, 3).reshape(C, T)
        outs.append(np.ascontiguousarray(oT.T).reshape(BL, N, C))
    return np.concatenate(outs, axis=0).astype(np.float32)


def kernel(x, w_qkv, w_proj, b_proj):
    nc = get_nc()
    in_maps = make_in_maps(x, w_qkv, w_proj, b_proj)
    res = bass_utils.run_bass_kernel_spmd(nc, in_maps, core_ids=list(range(NCORES)))
    return assemble_out(res.results)


# revision 24
# speedup vs baseline: 1.0159x; 1.0159x over previous
"""Trainium2 Bass kernel for a 12-head attention block.

Problem (hardcoded): x [16, 1024, 768] f32, w_qkv [2304, 768], w_proj
[768, 768], b_proj [768].  out = proj(softmax(q k^T / sqrt(64)) v).

Sharding: pure data parallel over batch - 16 batches / 8 cores = 2
batches per core, no collectives.  All layout transposes happen on the
host: each core receives x^T per batch and produces out^T per batch.

v3 design (vs v2):
  * HEAD-PAIR CONCURRENT QK^T: the S^T matmuls have contraction K=64
    (head_dim), which occupies only half the 128-row PE array.  The PE
    runs matmuls with disjoint 32-row-groups CONCURRENTLY (measured
    2.24x on HW for adjacent K=64 MMs on rows 0-63 / 64-127).  Heads
    2m (rows 0-63 of the qk tile) and 2m+1 (rows 64-127) are emitted
    as adjacent matmul pairs into separate PSUM tiles, halving the
    effective S^T cost from 98304 to ~49152 cycles/batch.
  * per-jc PV for the even head (o accums in 2 dedicated PSUM banks);
    the odd head's exp outputs are buffered in SBUF and its PV runs as
    8-matmul accumulation chains scheduled like fillers during the next
    pair (sharing the 2 filler PSUM banks).
  * PSUM: 2x s-pair tiles [128,1024] (4 banks) + 2x o_A [65,512]
    (2 banks) + 2x shared filler/B-chain [128,512] (2 banks) = 8.
  * all matmul operands bf16 (k stationary fp8e4); PSUM f32.
  * cross-batch software pipelining as v2: batch b+1 qkv projection and
    batch b out-projection run as paced fillers inside attention
    windows so the PE never waits on the exp (ACT) latency.
"""

import numpy as np
from contextlib import ExitStack

import ml_dtypes

import concourse.bass as bass
import concourse.mybir as mybir
import concourse.tile as tile
from concourse import bacc
from concourse import bass_utils

F32 = mybir.dt.float32
BF16 = mybir.dt.bfloat16
FP8 = mybir.dt.float8e4
EXP = mybir.ActivationFunctionType.Exp

B, N, C = 16, 1024, 768
H, D = 12, 64
E = 3 * C
NCORES = 8
BL = B // NCORES          # batches per core
T = BL * N                # tokens per core
KC = C // 128             # feature chunks of 128
JC = N // 128             # token chunks of 128
SCALE = float(D) ** -0.5

_CACHE = {}


def _mm(nc, out, lhsT, rhs, **kw):
    nc.tensor.matmul(out, lhsT=lhsT, rhs=rhs, **kw)


def _build(ctx, tc):
    nc = tc.nc
    dram = ctx.enter_context(tc.tile_pool(name="dram", bufs=1, space="DRAM"))
    # x^T blocked: [kc, b, 128, N] bf16, each per-batch chunk contiguous
    xT_d = dram.tile([KC, BL, 128, N], BF16, kind="ExternalInput", name="xTb", uniquify=False)
    # q/k weights j-major: [j, p, kc*128+c] so one j-output-chunk (used by
    # one qk_unit across all kc) is a single contiguous-row DMA
    wqkj_d = dram.tile([2 * KC, 128, C], BF16, kind="ExternalInput", name="wqkjb", uniquify=False)
    # v weights per-kc slabs [kc, 128, 768] bf16
    wv_d = dram.tile([KC, 128, C], BF16, kind="ExternalInput", name="wvb", uniquify=False)
    # w_proj^T per-kc slabs [kc, 128, 768] bf16
    wproj_d = dram.tile([KC, 128, C], BF16, kind="ExternalInput", name="wprojb", uniquify=False)
    bproj_d = dram.tile([C, 1], F32, kind="ExternalInput", name="bproj", uniquify=False)
    ident_d = dram.tile([128, 128], BF16, kind="ExternalInput", name="ident", uniquify=False)
    # out^T blocked: [oc, b, 128, N] f32
    outT_d = dram.tile([KC, BL, 128, N], BF16, kind="ExternalOutput", name="outTb", uniquify=False)

    consts = ctx.enter_context(tc.tile_pool(name="consts", bufs=1))
    wqk_pool = ctx.enter_context(tc.tile_pool(name="wqk", bufs=2 * (KC - 1)))
    wqk0_pool = ctx.enter_context(tc.tile_pool(name="wqk0", bufs=2 * KC))
    wv_pool = ctx.enter_context(tc.tile_pool(name="wv", bufs=KC))
    wp_pool = ctx.enter_context(tc.tile_pool(name="wproj", bufs=KC))
    xt_pool = ctx.enter_context(tc.tile_pool(name="xt", bufs=2 * KC))
    q_pool = ctx.enter_context(tc.tile_pool(name="qp", bufs=KC + 2))
    k_pool = ctx.enter_context(tc.tile_pool(name="kp", bufs=KC + 2))
    va_pool = ctx.enter_context(tc.tile_pool(name="va", bufs=2 * JC))
    ot_pool = ctx.enter_context(tc.tile_pool(name="ot", bufs=2 * KC))
    pp_pool = ctx.enter_context(tc.tile_pool(name="pp", bufs=21))
    sm_pool = ctx.enter_context(tc.tile_pool(name="small", bufs=8))
    lb_pool = ctx.enter_context(tc.tile_pool(name="lb", bufs=4))
    ob_pool = ctx.enter_context(tc.tile_pool(name="ob", bufs=2))
    # PSUM: s tiles are hf-interleaved head pairs [A-half | B-half]
    # [128, 1024] (2 banks each, 2 bufs): the two MMs writing one tile
    # are gated on the SAME recycle event (-> atomic concurrent pair)
    # while the two tiles double-buffer across i-halves/slots (-> the
    # exp latency pipelines).  + o_A accums (2 x 1 bank) + shared
    # filler / B-chain pool (2 x 1 bank) = 8 banks.
    ps_s = ctx.enter_context(tc.tile_pool(name="ps_s", bufs=2, space="PSUM"))
    po_pool = ctx.enter_context(tc.tile_pool(name="po", bufs=2, space="PSUM"))
    ps_a = ctx.enter_context(tc.tile_pool(name="ps_a", bufs=2, space="PSUM"))

    bias_sb = consts.tile([128, KC], F32)
    nc.sync.dma_start(
        out=bias_sb, in_=bproj_d[:, 0].rearrange("(k p) -> p k", p=128)
    )
    ident_sb = consts.tile([128, 128], BF16)
    nc.sync.dma_start(out=ident_sb, in_=ident_d)

    # ---- input DMA, ordered by first use ----
    xt = {}   # (b, kc) -> [128, N] bf16
    wqk_t, wv = {}, []

    def load_wqkj(j, split=1):
        t = wqk_pool.tile([128, KC, 128], BF16, name=f"wqkj{j}", tag="wqk")
        step = 128 // split
        for p in range(0, 128, step):
            nc.sync.dma_start(
                out=t[p:p + step, :, :].rearrange("p a b -> p (a b)"),
                in_=wqkj_d[j, p:p + step, :])
        wqk_t[j] = t

    # mt=0 k/q weights as per-kc tiles interleaved with batch-0 x chunks:
    # the first qk accumulation chain starts ~2us in (DMA-paced) instead
    # of waiting ~13us for monolithic weight tiles.
    wqk0 = {}
    for kc in range(KC):
        t = wqk0_pool.tile([128, 128], BF16, name=f"wqk0k_{kc}", tag="w0")
        nc.sync.dma_start(out=t, in_=wqkj_d[KC, :, kc * 128:(kc + 1) * 128])
        wqk0[(KC, kc)] = t
        xt[(0, kc)] = xt_pool.tile([128, N], BF16, name=f"xt0_{kc}", tag="xt")
        nc.sync.dma_start(out=xt[(0, kc)], in_=xT_d[kc, 0])
        t = wqk0_pool.tile([128, 128], BF16, name=f"wqk0q_{kc}", tag="w0")
        nc.sync.dma_start(out=t, in_=wqkj_d[0, :, kc * 128:(kc + 1) * 128])
        wqk0[(0, kc)] = t
    for kc in range(KC):
        wvt = wv_pool.tile([128, KC, 128], BF16, name=f"wv{kc}", tag="wv")
        nc.sync.dma_start(out=wvt.rearrange("p a b -> p (a b)"), in_=wv_d[kc])
        wv.append(wvt)
    for mt in range(1, KC):
        load_wqkj(KC + mt)
        load_wqkj(mt)
    wqk = {(j, kc): wqk_t[j][:, kc, :]
           for j in list(range(1, KC)) + list(range(KC + 1, 2 * KC))
           for kc in range(KC)}
    wqk.update(wqk0)
    wp = {}
    for kc in range(KC):
        t = wp_pool.tile([128, KC, 128], BF16, name=f"wpk{kc}", tag="wp")
        nc.sync.dma_start(out=t.rearrange("p a b -> p (a b)"), in_=wproj_d[kc])
        for oc in range(KC):
            wp[(kc, oc)] = t[:, oc, :]
    # batch-1 x after the weights
    for kc in range(KC):
        xt[(1, kc)] = xt_pool.tile([128, N], BF16, name=f"xt1_{kc}", tag="xt")
        nc.sync.dma_start(out=xt[(1, kc)], in_=xT_d[kc, 1])

    qt = {}   # (b, mt) -> [128, N] bf16   q features, head-pair mt
    kt = {}   # (b, mt) -> [128, N] fp8
    va = {}   # (b, jc) -> [128, H, D+1] bf16
    ot = {}   # (b, mt) -> [128, N] bf16

    # ---- phase A units: qkv projection for batch b, as closure LISTS
    # (sub-unit granularity ~1-1.3us for smooth filler pacing) ----
    def phase_a_units(b):
        units = {}

        def v_unit(jc):
            vat_box = {}

            def get_vat():
                if "t" not in vat_box:
                    vat_box["t"] = va_pool.tile(
                        [128, H, D + 1], BF16, name=f"va{b}_{jc}", tag="va")
                    va[(b, jc)] = vat_box["t"]
                return vat_box["t"]

            def emit_a():
                vat = get_vat()
                xs = [xt[(b, kc)][:, jc * 128:(jc + 1) * 128] for kc in range(KC)]
                wvf = [wv[kc].rearrange("p a b -> p (a b)") for kc in range(KC)]
                vps_a = ps_a.tile([128, 512], F32, name=f"vpsa{b}_{jc}", tag="a")
                for kc in range(KC):
                    _mm(nc, vps_a, xs[kc], wvf[kc][:, 0:512],
                        start=(kc == 0), stop=(kc == KC - 1))
                nc.vector.tensor_copy(
                    out=vat[:, 0:8, 0:D],
                    in_=vps_a.rearrange("p (h d) -> p h d", h=8),
                )

            def emit_b():
                vat = get_vat()
                xs = [xt[(b, kc)][:, jc * 128:(jc + 1) * 128] for kc in range(KC)]
                wvf = [wv[kc].rearrange("p a b -> p (a b)") for kc in range(KC)]
                vps_b = ps_a.tile([128, 512], F32, name=f"vpsb{b}_{jc}", tag="a")
                for kc in range(KC):
                    _mm(nc, vps_b[:, 0:256], xs[kc], wvf[kc][:, 512:C],
                        start=(kc == 0), stop=(kc == KC - 1))
                nc.vector.tensor_copy(
                    out=vat[:, 8:H, 0:D],
                    in_=vps_b[:, 0:256].rearrange("p (h d) -> p h d", h=4),
                )
                nc.vector.memset(vat[:, :, D:D + 1], 1.0)
            return [emit_a, emit_b]

        def qk_unit(which, mt):
            dest_box = {}

            def get_dest():
                if "t" not in dest_box:
                    # k (which=1) is only a stationary operand of QK^T:
                    # fp8e4 halves SBUF; perturbs logits ~0.6% on P.
                    if which == 1:
                        dest_box["t"] = k_pool.tile([128, N], FP8, name=f"k{b}_{mt}", tag="kp")
                        kt[(b, mt)] = dest_box["t"]
                    else:
                        dest_box["t"] = q_pool.tile([128, N], BF16, name=f"q{b}_{mt}", tag="qp")
                        qt[(b, mt)] = dest_box["t"]
                return dest_box["t"]

            def emit_hf(hf):
                def emit():
                    dest = get_dest()
                    ps = ps_a.tile([128, 512], F32, name=f"ps{b}_{which}_{mt}_{hf}", tag="a")
                    for kc in range(KC):
                        w = wqk[(which * KC + mt, kc)]
                        _mm(nc, ps,
                            w, xt[(b, kc)][:, hf * 512:(hf + 1) * 512],
                            start=(kc == 0), stop=(kc == KC - 1))
                    nc.vector.tensor_copy(out=dest[:, hf * 512:(hf + 1) * 512], in_=ps)
                return emit
            return [emit_hf(0), emit_hf(1)]

        for jc in range(JC):
            units[("v", jc)] = v_unit(jc)
        for mt in range(KC):
            units[("k", mt)] = qk_unit(1, mt)
            units[("q", mt)] = qk_unit(0, mt)
        return units

    # ---- normalize one [65, 512] accumulator half into ot rows ----
    def norm_half(b, mt, off, hf, o_ps, tag):
        l_sb = sm_pool.tile([1, 512], F32, name=f"l{tag}", tag="sm")
        nc.vector.tensor_copy(out=l_sb, in_=o_ps[D:D + 1, :])
        nc.vector.reciprocal_approx_fast(out=l_sb, in_=l_sb)
        lb = lb_pool.tile([D, 512], F32, name=f"lb{tag}", tag="lb")
        nc.gpsimd.partition_broadcast(lb, l_sb, channels=D)
        nc.vector.tensor_mul(
            out=ot[(b, mt)][off:off + D, hf * 512:(hf + 1) * 512],
            in0=o_ps[0:D, :], in1=lb,
        )

    # ---- attention for batch b: head-pair loops with paced fillers ----
    # fillers: list of dicts {dl, nb, w, fn} (deadline point, not-before
    # point, PE-ns weight).  48 pace points per batch (pair mt x jc).
    def attention_pairs(b, fillers):
        fillers.sort(key=lambda f: f["dl"])
        total_w = sum(f["w"] for f in fillers)
        state = {"w": 0.0}
        npace = KC * JC
        # pair-boundary points need ~2.2us of filler coverage (the last
        # slot's exps gate EVERYTHING pair-local: next S, PV, chains),
        # mid-pair slots much less -> nonlinear pacing budget
        pw = [2.2 if p % JC == 0 else (1.6 if p % JC == 7 else 0.75)
              for p in range(npace)]
        cw = []
        acc = 0.0
        for p in range(npace):
            acc += pw[p]
            cw.append(acc)
        for p in range(npace):
            cw[p] *= total_w / acc

        def insert(item):
            # keep sorted by deadline
            i = 0
            while i < len(fillers) and fillers[i]["dl"] <= item["dl"]:
                i += 1
            fillers.insert(i, item)

        def pace(point):
            budget = cw[point]
            while True:
                pick = None
                for i, f in enumerate(fillers):
                    if f["nb"] <= point and (state["w"] < budget or f["dl"] <= point):
                        pick = i
                        break
                    if f["dl"] > point and state["w"] >= budget:
                        break
                if pick is None:
                    break
                f = fillers.pop(pick)
                state["w"] += f["w"]
                f["fn"]()

        def b_chain(mt, hf, pt_map):
            def emit():
                o_ps = ps_a.tile([D + 1, 512], F32, name=f"obch{b}_{mt}_{hf}", tag="a")
                for j in range(JC):
                    _mm(nc, o_ps,
                        va[(b, j)][:, 2 * mt + 1, :],
                        pt_map[(j, hf)][:, 512:N],
                        start=(j == 0), stop=(j == JC - 1))
                norm_half(b, mt, D, hf, o_ps, f"B{b}_{mt}_{hf}")
            return emit

        for mt in range(KC):
            hA = 2 * mt
            kt_t, qt_t = kt[(b, mt)], qt[(b, mt)]
            o_A = [po_pool.tile([D + 1, 512], F32, name=f"oA{b}_{mt}_{hf}", tag="ops")
                   for hf in range(2)]
            # batch 1 last pair: fillers are flushed by then, so the ps_a
            # banks are free to hold per-jc B accumulators -> no serial
            # chain+norm tail before the final out-projection
            o_B = None
            if b == 1 and mt == KC - 1:
                o_B = [ps_a.tile([D + 1, 512], F32, name=f"oB{b}_{hf}", tag="a")
                       for hf in range(2)]
            ot[(b, mt)] = ot_pool.tile([128, N], BF16, name=f"ot{b}_{mt}", tag="ot")
            pts = {}
            for jc in range(JC + 1):
                point = mt * JC + jc
                # fillers FIRST in the slot: the greedy tile scheduler then
                # spends PE time on them while ACT finishes exp_B(jc-1), so
                # both s tiles of the pair are ready when the PE reaches
                # them and the pair MMs stay adjacent (-> concurrent).
                pace(min(point, npace - 1))
                if jc < JC:
                    js = slice(jc * 128, (jc + 1) * 128)
                    for hf in range(2):
                        s_t = ps_s.tile([128, N], F32,
                                        name=f"s{b}_{mt}_{jc}_{hf}", tag="s")
                        hs = slice(hf * 512, (hf + 1) * 512)
                        _mm(nc, s_t[:, 0:512], kt_t[0:D, js], qt_t[0:D, hs])
                        _mm(nc, s_t[:, 512:N], kt_t[D:128, js], qt_t[D:128, hs])
                        pt = pp_pool.tile([128, N], BF16,
                                          name=f"pt{b}_{mt}_{jc}_{hf}", tag="pp")
                        nc.scalar.activation(out=pt, in_=s_t, func=EXP, scale=SCALE)
                        pts[(jc, hf)] = pt
                if jc > 0:
                    j = jc - 1
                    for hf in range(2):
                        _mm(nc, o_A[hf],
                            va[(b, j)][:, hA, :],
                            pts[(j, hf)][:, 0:512],
                            start=(j == 0), stop=(j == JC - 1))
                    if o_B is not None:
                        for hf in range(2):
                            _mm(nc, o_B[hf],
                                va[(b, j)][:, hA + 1, :],
                                pts[(j, hf)][:, 512:N],
                                start=(j == 0), stop=(j == JC - 1))
            # normalize even head (rows 0..63 of ot)
            for hf in range(2):
                norm_half(b, mt, 0, hf, o_A[hf], f"A{b}_{mt}_{hf}")
            # odd head PV: 2 accumulation chains over buffered pt_B,
            # scheduled as fillers during the next pair (tail: inline)
            if o_B is None:
                b_chain(mt, 0, pts)()
                b_chain(mt, 1, pts)()
            else:
                for hf in range(2):
                    norm_half(b, mt, D, hf, o_B[hf], f"B{b}_{mt}_{hf}")
        # drain any remaining fillers
        for f in fillers:
            f["fn"]()

    # ---- output projection units for batch b ----
    def outproj_units(b):
        units = []
        obs = {}

        def half_unit(oc, hf):
            def emit():
                if oc not in obs:
                    obs[oc] = ob_pool.tile([128, N], BF16, name=f"ob{b}_{oc}", tag="ob")
                pps = ps_a.tile([128, 512], F32, name=f"pps{b}_{oc}_{hf}", tag="a")
                for kc in range(KC):
                    _mm(nc, pps,
                        wp[(kc, oc)],
                        ot[(b, kc)][:, hf * 512:(hf + 1) * 512],
                        start=(kc == 0), stop=(kc == KC - 1))
                nc.vector.tensor_scalar_add(
                    out=obs[oc][:, hf * 512:(hf + 1) * 512],
                    in0=pps, scalar1=bias_sb[:, oc:oc + 1])
                if hf == 1:
                    nc.sync.dma_start(out=outT_d[oc, b], in_=obs[oc])
            return emit

        for oc in range(KC):
            for hf in range(2):
                units.append(half_unit(oc, hf))
        return units

    # two-stage out-projection for batch 1: partial (kc 0..3) runs as
    # window-1 fillers once those head-pairs are normalized; the tail
    # only does the kc4+kc5 accumulation + fused (partial + bias) add.
    KPART = 4

    def outproj1_partials():
        parts = {}
        units = []

        def part_unit(oc):
            def emit():
                pt = xt_pool.tile([128, N], BF16, name=f"op1p{oc}", tag="xt")
                parts[oc] = pt
                for hf in range(2):
                    pps = ps_a.tile([128, 512], F32, name=f"pp1{oc}_{hf}", tag="a")
                    for kc in range(KPART):
                        _mm(nc, pps,
                            wp[(kc, oc)],
                            ot[(1, kc)][:, hf * 512:(hf + 1) * 512],
                            start=(kc == 0), stop=(kc == KPART - 1))
                    nc.vector.tensor_copy(
                        out=pt[:, hf * 512:(hf + 1) * 512], in_=pps)
            return emit

        for oc in range(KC):
            units.append(part_unit(oc))
        return parts, units

    def outproj1_finals(parts):
        for oc in range(KC):
            ob = ob_pool.tile([128, N], BF16, name=f"ob1f{oc}", tag="ob")
            for hf in range(2):
                pps = ps_s.tile([128, 512], F32, name=f"pf1{oc}_{hf}", tag="s")
                for kc in range(KPART, KC):
                    _mm(nc, pps,
                        wp[(kc, oc)],
                        ot[(1, kc)][:, hf * 512:(hf + 1) * 512],
                        start=(kc == KPART), stop=False)
                # partial re-enters via an identity matmul accumulation
                _mm(nc, pps, ident_sb,
                    parts[oc][:, hf * 512:(hf + 1) * 512],
                    start=False, stop=True)
                nc.vector.tensor_scalar_add(
                    out=ob[:, hf * 512:(hf + 1) * 512],
                    in0=pps, scalar1=bias_sb[:, oc:oc + 1])
            nc.sync.dma_start(out=outT_d[oc, 1], in_=ob)

    # ---- pipeline ----
    a0 = phase_a_units(0)
    a1 = phase_a_units(1)
    LAST = KC * JC - 1
    # prologue: minimum to start pair 0 + early v units
    for key in [("k", 0), ("q", 0)] + [("v", jc) for jc in range(3)]:
        for sub in a0[key]:
            sub()
    fill0 = []

    def add(fl, key, units, dl, nb, w):
        for sub in units[key]:
            fl.append({"dl": dl, "nb": nb, "w": w, "fn": sub})

    for jc in range(3, JC):
        add(fill0, ("v", jc), a0, jc - 1, 0, 960)
    for mt in range(1, KC):
        dl = mt * JC - 1
        add(fill0, ("k", mt), a0, dl, 0, 1280)
        add(fill0, ("q", mt), a0, dl, 0, 1280)
    for jc in range(3):
        add(fill0, ("v", jc), a1, 41 + jc, 30, 960)
    for jc in range(3, 6):
        add(fill0, ("v", jc), a1, 30 + 3 * (jc - 3), 26, 960)
    for w_ in ("k", "q"):
        add(fill0, (w_, 0), a1, 45, 30, 1280)
    attention_pairs(0, fill0)

    fill1 = []
    for jc in range(6, JC):
        add(fill1, ("v", jc), a1, jc - 1, 0, 960)
    for mt in range(1, KC):
        # staggered so completion precedes pair mt's first use
        ksub = a1[("k", mt)]
        qsub = a1[("q", mt)]
        fill1.append({"dl": mt * JC - 4, "nb": 0, "w": 1280, "fn": qsub[0]})
        fill1.append({"dl": mt * JC - 4, "nb": 0, "w": 1280, "fn": qsub[1]})
        fill1.append({"dl": mt * JC - 4, "nb": 0, "w": 1280, "fn": ksub[0]})
        fill1.append({"dl": min(mt * JC + 1, 39), "nb": 0, "w": 1280, "fn": ksub[1]})
    for i, u in enumerate(outproj_units(0)):
        oc = i // 2
        if oc % 2 == 0:
            # pinned near a pair boundary: guaranteed PE food while the
            # last exps of that pair gate all pair-local work
            dl = (oc + 1) * JC - 1 + (i % 2)
            fill1.append({"dl": dl, "nb": max(dl - 4, 0), "w": 1280, "fn": u})
        else:
            fill1.append({"dl": 10 + 4 * oc + 2 * (i % 2), "nb": 0, "w": 1280, "fn": u})
    op1_parts, op1_units = outproj1_partials()
    for i, u in enumerate(op1_units):
        fill1.append({"dl": min(33 + i, 39), "nb": 33, "w": 2150, "fn": u})
    attention_pairs(1, fill1)
    outproj1_finals(op1_parts)


def get_nc():
    if "nc" not in _CACHE:
        nc = bacc.Bacc(None, target_bir_lowering=False, debug=False)
        with tile.TileContext(nc) as tc:
            with ExitStack() as ctx:
                _build(ctx, tc)
        nc.compile()
        _CACHE["nc"] = nc
    return _CACHE["nc"]


def make_in_maps(x, w_qkv, w_proj, b_proj):
    x = np.asarray(x, dtype=np.float32)
    w_qkv = np.asarray(w_qkv, dtype=np.float32)
    w_proj = np.asarray(w_proj, dtype=np.float32)
    # q/k weights j-major: wqkjb[j, p, kc*128+c2] = w_qkv[j*128+c2, kc*128+p]
    wqkj = np.ascontiguousarray(
        w_qkv[0:2 * C].reshape(2 * KC, 128, KC, 128).transpose(0, 3, 2, 1)
        .reshape(2 * KC, 128, C)
    ).astype(ml_dtypes.bfloat16)
    # v weights per-kc slabs [kc, 128p, 768]: w_qkv^T[c, 2C:3C]
    wvb = np.ascontiguousarray(
        w_qkv[2 * C:3 * C].T.reshape(KC, 128, C)
    ).astype(ml_dtypes.bfloat16)
    # w_proj^T [c, o] -> per-kc slabs [kc, 128, 768] bf16
    wprojb = np.ascontiguousarray(w_proj.T.reshape(KC, 128, C)).astype(ml_dtypes.bfloat16)
    bp = np.ascontiguousarray(b_proj.astype(np.float32).reshape(C, 1))
    ident = np.eye(128, dtype=np.float32).astype(ml_dtypes.bfloat16)
    in_maps = []
    for c in range(NCORES):
        # x^T [c, t] -> blocks [kc, b, 128, N] bf16
        xT = x[c * BL:(c + 1) * BL].reshape(T, C).T  # [768, 2048]
        xb = np.ascontiguousarray(
            xT.reshape(KC, 128, BL, N).transpose(0, 2, 1, 3)
        ).astype(ml_dtypes.bfloat16)
        in_maps.append({"xTb": xb, "wqkjb": wqkj, "wvb": wvb,
                        "wprojb": wprojb, "bproj": bp, "ident": ident})
    return in_maps


def assemble_out(results):
    outs = []
    for c in range(NCORES):
        ob = results[c]["outTb"].astype(np.float32)  # [oc, b, 128, N]
        oT = ob.transpose(0, 2, 1, 3).reshape(C, T)
        outs.append(np.ascontiguousarray(oT.T).reshape(BL, N, C))
    return np.concatenate(outs, axis=0).astype(np.float32)


def kernel(x, w_qkv, w_proj, b_proj):
    nc = get_nc()
    in_maps = make_in_maps(x, w_qkv, w_proj, b_proj)
    res = bass_utils.run_bass_kernel_spmd(nc, in_maps, core_ids=list(range(NCORES)))
    return assemble_out(res.results)


# revision 25
# speedup vs baseline: 1.0237x; 1.0077x over previous
"""Trainium2 Bass kernel for a 12-head attention block.

Problem (hardcoded): x [16, 1024, 768] f32, w_qkv [2304, 768], w_proj
[768, 768], b_proj [768].  out = proj(softmax(q k^T / sqrt(64)) v).

Sharding: pure data parallel over batch - 16 batches / 8 cores = 2
batches per core, no collectives.  All layout transposes happen on the
host: each core receives x^T per batch and produces out^T per batch.

v3 design (vs v2):
  * HEAD-PAIR CONCURRENT QK^T: the S^T matmuls have contraction K=64
    (head_dim), which occupies only half the 128-row PE array.  The PE
    runs matmuls with disjoint 32-row-groups CONCURRENTLY (measured
    2.24x on HW for adjacent K=64 MMs on rows 0-63 / 64-127).  Heads
    2m (rows 0-63 of the qk tile) and 2m+1 (rows 64-127) are emitted
    as adjacent matmul pairs into separate PSUM tiles, halving the
    effective S^T cost from 98304 to ~49152 cycles/batch.
  * per-jc PV for the even head (o accums in 2 dedicated PSUM banks);
    the odd head's exp outputs are buffered in SBUF and its PV runs as
    8-matmul accumulation chains scheduled like fillers during the next
    pair (sharing the 2 filler PSUM banks).
  * PSUM: 2x s-pair tiles [128,1024] (4 banks) + 2x o_A [65,512]
    (2 banks) + 2x shared filler/B-chain [128,512] (2 banks) = 8.
  * all matmul operands bf16 (k stationary fp8e4); PSUM f32.
  * cross-batch software pipelining as v2: batch b+1 qkv projection and
    batch b out-projection run as paced fillers inside attention
    windows so the PE never waits on the exp (ACT) latency.
"""

import numpy as np
from contextlib import ExitStack

import ml_dtypes

import concourse.bass as bass
import concourse.mybir as mybir
import concourse.tile as tile
from concourse import bacc
from concourse import bass_utils

F32 = mybir.dt.float32
BF16 = mybir.dt.bfloat16
FP8 = mybir.dt.float8e4
EXP = mybir.ActivationFunctionType.Exp

B, N, C = 16, 1024, 768
H, D = 12, 64
E = 3 * C
NCORES = 8
BL = B // NCORES          # batches per core
T = BL * N                # tokens per core
KC = C // 128             # feature chunks of 128
JC = N // 128             # token chunks of 128
SCALE = float(D) ** -0.5

_CACHE = {}


def _mm(nc, out, lhsT, rhs, **kw):
    nc.tensor.matmul(out, lhsT=lhsT, rhs=rhs, **kw)


def _build(ctx, tc):
    nc = tc.nc
    dram = ctx.enter_context(tc.tile_pool(name="dram", bufs=1, space="DRAM"))
    # x^T blocked: [kc, b, 128, N] bf16, each per-batch chunk contiguous
    xT_d = dram.tile([KC, BL, 128, N], BF16, kind="ExternalInput", name="xTb", uniquify=False)
    # q/k weights j-major: [j, p, kc*128+c] so one j-output-chunk (used by
    # one qk_unit across all kc) is a single contiguous-row DMA
    wqkj_d = dram.tile([2 * KC, 128, C], BF16, kind="ExternalInput", name="wqkjb", uniquify=False)
    # v weights per-kc slabs [kc, 128, 768] bf16
    wv_d = dram.tile([KC, 128, C], BF16, kind="ExternalInput", name="wvb", uniquify=False)
    # w_proj^T per-kc slabs [kc, 128, 768] bf16
    wproj_d = dram.tile([KC, 128, C], BF16, kind="ExternalInput", name="wprojb", uniquify=False)
    bproj_d = dram.tile([C, 1], F32, kind="ExternalInput", name="bproj", uniquify=False)
    ident_d = dram.tile([128, 128], BF16, kind="ExternalInput", name="ident", uniquify=False)
    # out^T blocked: [oc, b, 128, N] f32
    outT_d = dram.tile([KC, BL, 128, N], BF16, kind="ExternalOutput", name="outTb", uniquify=False)

    consts = ctx.enter_context(tc.tile_pool(name="consts", bufs=1))
    wqk_pool = ctx.enter_context(tc.tile_pool(name="wqk", bufs=2 * (KC - 1)))
    wqk0_pool = ctx.enter_context(tc.tile_pool(name="wqk0", bufs=2 * KC))
    wv_pool = ctx.enter_context(tc.tile_pool(name="wv", bufs=KC))
    wp_pool = ctx.enter_context(tc.tile_pool(name="wproj", bufs=KC))
    xt_pool = ctx.enter_context(tc.tile_pool(name="xt", bufs=2 * KC))
    q_pool = ctx.enter_context(tc.tile_pool(name="qp", bufs=KC + 2))
    k_pool = ctx.enter_context(tc.tile_pool(name="kp", bufs=KC + 2))
    va_pool = ctx.enter_context(tc.tile_pool(name="va", bufs=2 * JC))
    ot_pool = ctx.enter_context(tc.tile_pool(name="ot", bufs=2 * KC))
    pp_pool = ctx.enter_context(tc.tile_pool(name="pp", bufs=21))
    sm_pool = ctx.enter_context(tc.tile_pool(name="small", bufs=8))
    lb_pool = ctx.enter_context(tc.tile_pool(name="lb", bufs=4))
    ob_pool = ctx.enter_context(tc.tile_pool(name="ob", bufs=2))
    # PSUM: s tiles are hf-interleaved head pairs [A-half | B-half]
    # [128, 1024] (2 banks each, 2 bufs): the two MMs writing one tile
    # are gated on the SAME recycle event (-> atomic concurrent pair)
    # while the two tiles double-buffer across i-halves/slots (-> the
    # exp latency pipelines).  + o_A accums (2 x 1 bank) + shared
    # filler / B-chain pool (2 x 1 bank) = 8 banks.
    ps_s = ctx.enter_context(tc.tile_pool(name="ps_s", bufs=2, space="PSUM"))
    po_pool = ctx.enter_context(tc.tile_pool(name="po", bufs=2, space="PSUM"))
    ps_a = ctx.enter_context(tc.tile_pool(name="ps_a", bufs=2, space="PSUM"))

    bias_sb = consts.tile([128, KC], F32)
    nc.sync.dma_start(
        out=bias_sb, in_=bproj_d[:, 0].rearrange("(k p) -> p k", p=128)
    )
    ident_sb = consts.tile([128, 128], BF16)
    nc.sync.dma_start(out=ident_sb, in_=ident_d)

    # ---- input DMA, ordered by first use ----
    xt = {}   # (b, kc) -> [128, N] bf16
    wqk_t, wv = {}, []

    def load_wqkj(j, split=1):
        t = wqk_pool.tile([128, KC, 128], BF16, name=f"wqkj{j}", tag="wqk")
        step = 128 // split
        for p in range(0, 128, step):
            nc.sync.dma_start(
                out=t[p:p + step, :, :].rearrange("p a b -> p (a b)"),
                in_=wqkj_d[j, p:p + step, :])
        wqk_t[j] = t

    # mt=0 k/q weights as per-kc tiles interleaved with batch-0 x chunks:
    # the first qk accumulation chain starts ~2us in (DMA-paced) instead
    # of waiting ~13us for monolithic weight tiles.
    wqk0 = {}
    for kc in range(KC):
        t = wqk0_pool.tile([128, 128], BF16, name=f"wqk0k_{kc}", tag="w0")
        nc.sync.dma_start(out=t, in_=wqkj_d[KC, :, kc * 128:(kc + 1) * 128])
        wqk0[(KC, kc)] = t
        xt[(0, kc)] = xt_pool.tile([128, N], BF16, name=f"xt0_{kc}", tag="xt")
        nc.sync.dma_start(out=xt[(0, kc)], in_=xT_d[kc, 0])
        t = wqk0_pool.tile([128, 128], BF16, name=f"wqk0q_{kc}", tag="w0")
        nc.sync.dma_start(out=t, in_=wqkj_d[0, :, kc * 128:(kc + 1) * 128])
        wqk0[(0, kc)] = t
    for kc in range(KC):
        wvt = wv_pool.tile([128, KC, 128], BF16, name=f"wv{kc}", tag="wv")
        nc.sync.dma_start(out=wvt.rearrange("p a b -> p (a b)"), in_=wv_d[kc])
        wv.append(wvt)
    for mt in range(1, KC):
        load_wqkj(KC + mt)
        load_wqkj(mt)
    wqk = {(j, kc): wqk_t[j][:, kc, :]
           for j in list(range(1, KC)) + list(range(KC + 1, 2 * KC))
           for kc in range(KC)}
    wqk.update(wqk0)
    wp = {}
    for kc in range(KC):
        t = wp_pool.tile([128, KC, 128], BF16, name=f"wpk{kc}", tag="wp")
        nc.sync.dma_start(out=t.rearrange("p a b -> p (a b)"), in_=wproj_d[kc])
        for oc in range(KC):
            wp[(kc, oc)] = t[:, oc, :]
    # batch-1 x after the weights
    for kc in range(KC):
        xt[(1, kc)] = xt_pool.tile([128, N], BF16, name=f"xt1_{kc}", tag="xt")
        nc.sync.dma_start(out=xt[(1, kc)], in_=xT_d[kc, 1])

    qt = {}   # (b, mt) -> [128, N] bf16   q features, head-pair mt
    kt = {}   # (b, mt) -> [128, N] fp8
    va = {}   # (b, jc) -> [128, H, D+1] bf16
    ot = {}   # (b, mt) -> [128, N] bf16

    # ---- phase A units: qkv projection for batch b, as closure LISTS
    # (sub-unit granularity ~1-1.3us for smooth filler pacing) ----
    def phase_a_units(b):
        units = {}

        def v_unit(jc):
            vat_box = {}

            def get_vat():
                if "t" not in vat_box:
                    vat_box["t"] = va_pool.tile(
                        [128, H, D + 1], BF16, name=f"va{b}_{jc}", tag="va")
                    va[(b, jc)] = vat_box["t"]
                return vat_box["t"]

            def emit_a():
                vat = get_vat()
                xs = [xt[(b, kc)][:, jc * 128:(jc + 1) * 128] for kc in range(KC)]
                wvf = [wv[kc].rearrange("p a b -> p (a b)") for kc in range(KC)]
                vps_a = ps_a.tile([128, 512], F32, name=f"vpsa{b}_{jc}", tag="a")
                for kc in range(KC):
                    _mm(nc, vps_a, xs[kc], wvf[kc][:, 0:512],
                        start=(kc == 0), stop=(kc == KC - 1))
                nc.vector.tensor_copy(
                    out=vat[:, 0:8, 0:D],
                    in_=vps_a.rearrange("p (h d) -> p h d", h=8),
                )

            def emit_b():
                vat = get_vat()
                xs = [xt[(b, kc)][:, jc * 128:(jc + 1) * 128] for kc in range(KC)]
                wvf = [wv[kc].rearrange("p a b -> p (a b)") for kc in range(KC)]
                vps_b = ps_a.tile([128, 512], F32, name=f"vpsb{b}_{jc}", tag="a")
                for kc in range(KC):
                    _mm(nc, vps_b[:, 0:256], xs[kc], wvf[kc][:, 512:C],
                        start=(kc == 0), stop=(kc == KC - 1))
                nc.vector.tensor_copy(
                    out=vat[:, 8:H, 0:D],
                    in_=vps_b[:, 0:256].rearrange("p (h d) -> p h d", h=4),
                )
                nc.vector.memset(vat[:, :, D:D + 1], 1.0)
            return [emit_a, emit_b]

        def qk_unit(which, mt):
            dest_box = {}

            def get_dest():
                if "t" not in dest_box:
                    # k (which=1) is only a stationary operand of QK^T:
                    # fp8e4 halves SBUF; perturbs logits ~0.6% on P.
                    if which == 1:
                        dest_box["t"] = k_pool.tile([128, N], FP8, name=f"k{b}_{mt}", tag="kp")
                        kt[(b, mt)] = dest_box["t"]
                    else:
                        dest_box["t"] = q_pool.tile([128, N], BF16, name=f"q{b}_{mt}", tag="qp")
                        qt[(b, mt)] = dest_box["t"]
                return dest_box["t"]

            def emit_hf(hf):
                def emit():
                    dest = get_dest()
                    ps = ps_a.tile([128, 512], F32, name=f"ps{b}_{which}_{mt}_{hf}", tag="a")
                    for kc in range(KC):
                        w = wqk[(which * KC + mt, kc)]
                        _mm(nc, ps,
                            w, xt[(b, kc)][:, hf * 512:(hf + 1) * 512],
                            start=(kc == 0), stop=(kc == KC - 1))
                    nc.vector.tensor_copy(out=dest[:, hf * 512:(hf + 1) * 512], in_=ps)
                return emit
            return [emit_hf(0), emit_hf(1)]

        for jc in range(JC):
            units[("v", jc)] = v_unit(jc)
        for mt in range(KC):
            units[("k", mt)] = qk_unit(1, mt)
            units[("q", mt)] = qk_unit(0, mt)
        return units

    # ---- normalize one [65, 512] accumulator half into ot rows ----
    def norm_half(b, mt, off, hf, o_ps, tag):
        l_sb = sm_pool.tile([1, 512], F32, name=f"l{tag}", tag="sm")
        nc.vector.tensor_copy(out=l_sb, in_=o_ps[D:D + 1, :])
        nc.vector.reciprocal_approx_fast(out=l_sb, in_=l_sb)
        lb = lb_pool.tile([D, 512], F32, name=f"lb{tag}", tag="lb")
        nc.gpsimd.partition_broadcast(lb, l_sb, channels=D)
        nc.vector.tensor_mul(
            out=ot[(b, mt)][off:off + D, hf * 512:(hf + 1) * 512],
            in0=o_ps[0:D, :], in1=lb,
        )

    # ---- attention for batch b: head-pair loops with paced fillers ----
    # fillers: list of dicts {dl, nb, w, fn} (deadline point, not-before
    # point, PE-ns weight).  48 pace points per batch (pair mt x jc).
    def attention_pairs(b, fillers):
        fillers.sort(key=lambda f: f["dl"])
        total_w = sum(f["w"] for f in fillers)
        state = {"w": 0.0}
        npace = KC * JC
        # pair-boundary points need ~2.2us of filler coverage (the last
        # slot's exps gate EVERYTHING pair-local: next S, PV, chains),
        # mid-pair slots much less -> nonlinear pacing budget
        pw = [2.2 if p % JC == 0 else (1.6 if p % JC == 7 else 0.75)
              for p in range(npace)]
        cw = []
        acc = 0.0
        for p in range(npace):
            acc += pw[p]
            cw.append(acc)
        for p in range(npace):
            cw[p] *= total_w / acc

        def insert(item):
            # keep sorted by deadline
            i = 0
            while i < len(fillers) and fillers[i]["dl"] <= item["dl"]:
                i += 1
            fillers.insert(i, item)

        def pace(point):
            budget = cw[point]
            while True:
                pick = None
                for i, f in enumerate(fillers):
                    if f["nb"] <= point and (state["w"] < budget or f["dl"] <= point):
                        pick = i
                        break
                    if f["dl"] > point and state["w"] >= budget:
                        break
                if pick is None:
                    break
                f = fillers.pop(pick)
                state["w"] += f["w"]
                f["fn"]()

        def b_chain(mt, hf, pt_map):
            def emit():
                o_ps = ps_a.tile([D + 1, 512], F32, name=f"obch{b}_{mt}_{hf}", tag="a")
                for j in range(JC):
                    _mm(nc, o_ps,
                        va[(b, j)][:, 2 * mt + 1, :],
                        pt_map[(j, hf)][:, 512:N],
                        start=(j == 0), stop=(j == JC - 1))
                norm_half(b, mt, D, hf, o_ps, f"B{b}_{mt}_{hf}")
            return emit

        for mt in range(KC):
            hA = 2 * mt
            kt_t, qt_t = kt[(b, mt)], qt[(b, mt)]
            o_A = [po_pool.tile([D + 1, 512], F32, name=f"oA{b}_{mt}_{hf}", tag="ops")
                   for hf in range(2)]
            # batch 1 last pair: fillers are flushed by then, so the ps_a
            # banks are free to hold per-jc B accumulators -> no serial
            # chain+norm tail before the final out-projection
            o_B = None
            if b == 1 and mt == KC - 1:
                o_B = [ps_a.tile([D + 1, 512], F32, name=f"oB{b}_{hf}", tag="a")
                       for hf in range(2)]
            ot[(b, mt)] = ot_pool.tile([128, N], BF16, name=f"ot{b}_{mt}", tag="ot")
            pts = {}
            for jc in range(JC + 1):
                point = mt * JC + jc
                if jc < JC:
                    js = slice(jc * 128, (jc + 1) * 128)
                    for hf in range(2):
                        s_t = ps_s.tile([128, N], F32,
                                        name=f"s{b}_{mt}_{jc}_{hf}", tag="s")
                        hs = slice(hf * 512, (hf + 1) * 512)
                        _mm(nc, s_t[:, 0:512], kt_t[0:D, js], qt_t[0:D, hs])
                        _mm(nc, s_t[:, 512:N], kt_t[D:128, js], qt_t[D:128, hs])
                        pt = pp_pool.tile([128, N], BF16,
                                          name=f"pt{b}_{mt}_{jc}_{hf}", tag="pp")
                        nc.scalar.activation(out=pt, in_=s_t, func=EXP, scale=SCALE)
                        pts[(jc, hf)] = pt
                # fillers AFTER this slot's S/exp emission: their priority
                # places them between this S pair and the next, so the PE
                # chews them while ACT finishes this slot's exps (pair
                # atomicity now comes from the shared s tile, not from
                # filler placement)
                pace(min(point, npace - 1))
                if jc > 0:
                    j = jc - 1
                    for hf in range(2):
                        _mm(nc, o_A[hf],
                            va[(b, j)][:, hA, :],
                            pts[(j, hf)][:, 0:512],
                            start=(j == 0), stop=(j == JC - 1))
                    if o_B is not None:
                        for hf in range(2):
                            _mm(nc, o_B[hf],
                                va[(b, j)][:, hA + 1, :],
                                pts[(j, hf)][:, 512:N],
                                start=(j == 0), stop=(j == JC - 1))
            # normalize even head (rows 0..63 of ot)
            for hf in range(2):
                norm_half(b, mt, 0, hf, o_A[hf], f"A{b}_{mt}_{hf}")
            # odd head PV: 2 accumulation chains over buffered pt_B,
            # scheduled as fillers during the next pair (tail: inline)
            if o_B is None:
                b_chain(mt, 0, pts)()
                b_chain(mt, 1, pts)()
            else:
                for hf in range(2):
                    norm_half(b, mt, D, hf, o_B[hf], f"B{b}_{mt}_{hf}")
        # drain any remaining fillers
        for f in fillers:
            f["fn"]()

    # ---- output projection units for batch b ----
    def outproj_units(b):
        units = []
        obs = {}

        def half_unit(oc, hf):
            def emit():
                if oc not in obs:
                    obs[oc] = ob_pool.tile([128, N], BF16, name=f"ob{b}_{oc}", tag="ob")
                pps = ps_a.tile([128, 512], F32, name=f"pps{b}_{oc}_{hf}", tag="a")
                for kc in range(KC):
                    _mm(nc, pps,
                        wp[(kc, oc)],
                        ot[(b, kc)][:, hf * 512:(hf + 1) * 512],
                        start=(kc == 0), stop=(kc == KC - 1))
                nc.vector.tensor_scalar_add(
                    out=obs[oc][:, hf * 512:(hf + 1) * 512],
                    in0=pps, scalar1=bias_sb[:, oc:oc + 1])
                if hf == 1:
                    nc.sync.dma_start(out=outT_d[oc, b], in_=obs[oc])
            return emit

        for oc in range(KC):
            for hf in range(2):
                units.append(half_unit(oc, hf))
        return units

    # two-stage out-projection for batch 1: partial (kc 0..3) runs as
    # window-1 fillers once those head-pairs are normalized; the tail
    # only does the kc4+kc5 accumulation + fused (partial + bias) add.
    KPART = 4

    def outproj1_partials():
        parts = {}
        units = []

        def part_unit(oc):
            def emit():
                pt = xt_pool.tile([128, N], BF16, name=f"op1p{oc}", tag="xt")
                parts[oc] = pt
                for hf in range(2):
                    pps = ps_a.tile([128, 512], F32, name=f"pp1{oc}_{hf}", tag="a")
                    for kc in range(KPART):
                        _mm(nc, pps,
                            wp[(kc, oc)],
                            ot[(1, kc)][:, hf * 512:(hf + 1) * 512],
                            start=(kc == 0), stop=(kc == KPART - 1))
                    nc.vector.tensor_copy(
                        out=pt[:, hf * 512:(hf + 1) * 512], in_=pps)
            return emit

        for oc in range(KC):
            units.append(part_unit(oc))
        return parts, units

    def outproj1_finals(parts):
        for oc in range(KC):
            ob = ob_pool.tile([128, N], BF16, name=f"ob1f{oc}", tag="ob")
            for hf in range(2):
                pps = ps_s.tile([128, 512], F32, name=f"pf1{oc}_{hf}", tag="s")
                for kc in range(KPART, KC):
                    _mm(nc, pps,
                        wp[(kc, oc)],
                        ot[(1, kc)][:, hf * 512:(hf + 1) * 512],
                        start=(kc == KPART), stop=False)
                # partial re-enters via an identity matmul accumulation
                _mm(nc, pps, ident_sb,
                    parts[oc][:, hf * 512:(hf + 1) * 512],
                    start=False, stop=True)
                nc.vector.tensor_scalar_add(
                    out=ob[:, hf * 512:(hf + 1) * 512],
                    in0=pps, scalar1=bias_sb[:, oc:oc + 1])
            nc.sync.dma_start(out=outT_d[oc, 1], in_=ob)

    # ---- pipeline ----
    a0 = phase_a_units(0)
    a1 = phase_a_units(1)
    LAST = KC * JC - 1
    # prologue: minimum to start pair 0 + early v units
    for key in [("k", 0), ("q", 0)] + [("v", jc) for jc in range(3)]:
        for sub in a0[key]:
            sub()
    fill0 = []

    def add(fl, key, units, dl, nb, w):
        for sub in units[key]:
            fl.append({"dl": dl, "nb": nb, "w": w, "fn": sub})

    for jc in range(3, JC):
        add(fill0, ("v", jc), a0, jc - 1, 0, 960)
    for mt in range(1, KC):
        dl = mt * JC - 1
        add(fill0, ("k", mt), a0, dl, 0, 1280)
        add(fill0, ("q", mt), a0, dl, 0, 1280)
    for jc in range(3):
        add(fill0, ("v", jc), a1, 41 + jc, 30, 960)
    for jc in range(3, 6):
        add(fill0, ("v", jc), a1, 30 + 3 * (jc - 3), 26, 960)
    for w_ in ("k", "q"):
        add(fill0, (w_, 0), a1, 45, 30, 1280)
    attention_pairs(0, fill0)

    fill1 = []
    for jc in range(6, JC):
        add(fill1, ("v", jc), a1, jc - 1, 0, 960)
    for mt in range(1, KC):
        # staggered so completion precedes pair mt's first use
        ksub = a1[("k", mt)]
        qsub = a1[("q", mt)]
        fill1.append({"dl": mt * JC - 4, "nb": 0, "w": 1280, "fn": qsub[0]})
        fill1.append({"dl": mt * JC - 4, "nb": 0, "w": 1280, "fn": qsub[1]})
        fill1.append({"dl": mt * JC - 4, "nb": 0, "w": 1280, "fn": ksub[0]})
        fill1.append({"dl": min(mt * JC + 1, 39), "nb": 0, "w": 1280, "fn": ksub[1]})
    for i, u in enumerate(outproj_units(0)):
        oc = i // 2
        if oc % 2 == 0:
            # pinned near a pair boundary: guaranteed PE food while the
            # last exps of that pair gate all pair-local work
            dl = (oc + 1) * JC + (i % 2)
            fill1.append({"dl": dl, "nb": max(dl - 4, 0), "w": 1280, "fn": u})
        else:
            fill1.append({"dl": 10 + 4 * oc + 2 * (i % 2), "nb": 0, "w": 1280, "fn": u})
    op1_parts, op1_units = outproj1_partials()
    for i, u in enumerate(op1_units):
        fill1.append({"dl": min(33 + i, 39), "nb": 33, "w": 2150, "fn": u})
    attention_pairs(1, fill1)
    outproj1_finals(op1_parts)


def get_nc():
    if "nc" not in _CACHE:
        nc = bacc.Bacc(None, target_bir_lowering=False, debug=False)
        with tile.TileContext(nc) as tc:
            with ExitStack() as ctx:
                _build(ctx, tc)
        nc.compile()
        _CACHE["nc"] = nc
    return _CACHE["nc"]


def make_in_maps(x, w_qkv, w_proj, b_proj):
    x = np.asarray(x, dtype=np.float32)
    w_qkv = np.asarray(w_qkv, dtype=np.float32)
    w_proj = np.asarray(w_proj, dtype=np.float32)
    # q/k weights j-major: wqkjb[j, p, kc*128+c2] = w_qkv[j*128+c2, kc*128+p]
    wqkj = np.ascontiguousarray(
        w_qkv[0:2 * C].reshape(2 * KC, 128, KC, 128).transpose(0, 3, 2, 1)
        .reshape(2 * KC, 128, C)
    ).astype(ml_dtypes.bfloat16)
    # v weights per-kc slabs [kc, 128p, 768]: w_qkv^T[c, 2C:3C]
    wvb = np.ascontiguousarray(
        w_qkv[2 * C:3 * C].T.reshape(KC, 128, C)
    ).astype(ml_dtypes.bfloat16)
    # w_proj^T [c, o] -> per-kc slabs [kc, 128, 768] bf16
    wprojb = np.ascontiguousarray(w_proj.T.reshape(KC, 128, C)).astype(ml_dtypes.bfloat16)
    bp = np.ascontiguousarray(b_proj.astype(np.float32).reshape(C, 1))
    ident = np.eye(128, dtype=np.float32).astype(ml_dtypes.bfloat16)
    in_maps = []
    for c in range(NCORES):
        # x^T [c, t] -> blocks [kc, b, 128, N] bf16
        xT = x[c * BL:(c + 1) * BL].reshape(T, C).T  # [768, 2048]
        xb = np.ascontiguousarray(
            xT.reshape(KC, 128, BL, N).transpose(0, 2, 1, 3)
        ).astype(ml_dtypes.bfloat16)
        in_maps.append({"xTb": xb, "wqkjb": wqkj, "wvb": wvb,
                        "wprojb": wprojb, "bproj": bp, "ident": ident})
    return in_maps


def assemble_out(results):
    outs = []
    for c in range(NCORES):
        ob = results[c]["outTb"].astype(np.float32)  # [oc, b, 128, N]
        oT = ob.transpose(0, 2, 1, 3).reshape(C, T)
        outs.append(np.ascontiguousarray(oT.T).reshape(BL, N, C))
    return np.concatenate(outs, axis=0).astype(np.float32)


def kernel(x, w_qkv, w_proj, b_proj):
    nc = get_nc()
    in_maps = make_in_maps(x, w_qkv, w_proj, b_proj)
    res = bass_utils.run_bass_kernel_spmd(nc, in_maps, core_ids=list(range(NCORES)))
    return assemble_out(res.results)
